# revision 9
# baseline (speedup 1.0000x reference)
"""Trainium2 Bass kernel for the gated-attention MIL pooling layer.

Computes, for x:[256,128,1024], v,u:[1024,512], w:[512,1]:
    h = tanh(x @ v); g = sigmoid(x @ u)
    scores = (h*g) @ w                      # [256,128,1]
    alpha  = softmax(scores, axis=0)        # over the 256 instances

Sharding: data-parallel over the batch axis (128 -> 16 per core, 8 cores).
Each core handles a [4096,1024]x[1024,512] matmul pair + a local softmax
(softmax is over instances, which live entirely on one core).

x is transposed host-side to [in_dim, m] so the Bass kernel can feed the
PE array without on-chip transposes (the contraction dim must sit on SBUF
partitions).  Matmuls run as float32r (full PE rate at moving dim >= 256,
~fp32 precision).

Written in raw Bass (explicit per-engine programs + semaphores): the
walrus build in this container rejects instructions carrying more than
one attached semaphore wait, which rules out Tile-generated sync.  All
waits here are standalone wait_ge instructions.
"""

import numpy as np

N_INST, BATCH, IN_DIM, L_DIM = 256, 128, 1024, 512
N_CORES = 8
B_LOC = BATCH // N_CORES            # 16 batch elements per core
M = N_INST * B_LOC                  # 4096 rows per core
P = 128                             # SBUF partitions
KO = IN_DIM // P                    # 8 contraction subtiles
MT = M // P                         # 32 m-tiles per core
MS = 4                              # m-tiles per x DMA chunk
NS = MT // MS                       # 8 DMA chunks

_CACHE = {}


def _build_bass():
    from contextlib import ExitStack

    import concourse.bass as bass
    import concourse.mybir as mybir

    f32 = mybir.dt.float32
    f32r = mybir.dt.float32r
    AF = mybir.ActivationFunctionType
    ALU = mybir.AluOpType

    nc = bass.Bass(
        trn_type="TRN2",
        target_bir_lowering=False,
        debug=False,
        enable_asserts=False,
    )

    xt = nc.dram_tensor("xt", [IN_DIM, M], f32r, kind="ExternalInput").ap()
    v = nc.dram_tensor("v", [IN_DIM, L_DIM], f32r, kind="ExternalInput").ap()
    u = nc.dram_tensor("u", [IN_DIM, L_DIM], f32r, kind="ExternalInput").ap()
    w_rep = nc.dram_tensor("w_rep", [P, L_DIM], f32, kind="ExternalInput").ap()
    sel = nc.dram_tensor("sel", [P, B_LOC], f32, kind="ExternalInput").ap()
    sel2 = nc.dram_tensor("sel2", [B_LOC, P], f32, kind="ExternalInput").ap()
    ident = nc.dram_tensor("ident", [P, P], f32, kind="ExternalInput").ap()
    out = nc.dram_tensor("out", [MT, P], f32, kind="ExternalOutput").ap()

    xt3 = xt.rearrange("(ko p) m -> p ko m", p=P)

    ctx = ExitStack()
    with ctx:
        v_sb = ctx.enter_context(nc.sbuf_tensor("v_sb", [P, KO, L_DIM], f32r))
        u_sb = ctx.enter_context(nc.sbuf_tensor("u_sb", [P, KO, L_DIM], f32r))
        w_sb = ctx.enter_context(nc.sbuf_tensor("w_sb", [P, L_DIM], f32))
        sel_sb = ctx.enter_context(nc.sbuf_tensor("sel_sb", [P, B_LOC], f32))
        sel2_sb = ctx.enter_context(nc.sbuf_tensor("sel2_sb", [B_LOC, P], f32))
        id_sb = ctx.enter_context(nc.sbuf_tensor("id_sb", [P, P], f32))
        x_sb = ctx.enter_context(nc.sbuf_tensor("x_sb", [P, 2, KO, MS * P], f32r))
        th_sb = ctx.enter_context(nc.sbuf_tensor("th_sb", [P, 2, L_DIM], f32))
        sg_sb = ctx.enter_context(nc.sbuf_tensor("sg_sb", [P, 2, L_DIM], f32))
        tw_sb = ctx.enter_context(nc.sbuf_tensor("tw_sb", [P, L_DIM], f32))
        z_sb = ctx.enter_context(nc.sbuf_tensor("z_sb", [P, L_DIM], f32))
        S_sb = ctx.enter_context(nc.sbuf_tensor("S_sb", [P, MT], f32))
        E_sb = ctx.enter_context(nc.sbuf_tensor("E_sb", [P, MT], f32))
        rsum_sb = ctx.enter_context(nc.sbuf_tensor("rsum_sb", [P, 1], f32))
        recip_sb = ctx.enter_context(nc.sbuf_tensor("recip_sb", [B_LOC, 1], f32))
        rep_sb = ctx.enter_context(nc.sbuf_tensor("rep_sb", [P, 1], f32))
        alpha_sb = ctx.enter_context(nc.sbuf_tensor("alpha_sb", [P, MT], f32))
        at_sb = ctx.enter_context(nc.sbuf_tensor("at_sb", [MT, P], f32))

        h_ps = ctx.enter_context(nc.psum_tensor("h_ps", [P, 2, L_DIM], f32))
        g_ps = ctx.enter_context(nc.psum_tensor("g_ps", [P, 2, L_DIM], f32))
        gs_ps = ctx.enter_context(nc.psum_tensor("gs_ps", [B_LOC, 1], f32))
        rep_ps = ctx.enter_context(nc.psum_tensor("rep_ps", [P, 1], f32))
        at_ps = ctx.enter_context(nc.psum_tensor("at_ps", [MT, P], f32))

        s_v = ctx.enter_context(nc.semaphore("s_v"))
        s_u = ctx.enter_context(nc.semaphore("s_u"))
        s_w = ctx.enter_context(nc.semaphore("s_w"))
        s_sel = ctx.enter_context(nc.semaphore("s_sel"))
        s_sel2 = ctx.enter_context(nc.semaphore("s_sel2"))
        s_id = ctx.enter_context(nc.semaphore("s_id"))
        s_x = [ctx.enter_context(nc.semaphore(f"s_x{i}")) for i in range(NS)]
        s_out = ctx.enter_context(nc.semaphore("s_out"))
        s_pe = ctx.enter_context(nc.semaphore("s_pe"))
        s_act = ctx.enter_context(nc.semaphore("s_act"))
        s_dve = ctx.enter_context(nc.semaphore("s_dve"))

        block = ctx.enter_context(nc.Block())

        # Engine tick conventions:
        #   s_pe : +1 after each finished matmul accumulation group
        #          (tile t: h-group -> 2t+1, g-group -> 2t+2); epilogue
        #          matmuls -> 65 (gs), 66 (rep), 67 (transpose)
        #   s_act: tanh of tile t -> 2t+1, sigmoid -> 2t+2; exp -> 65
        #   s_dve: tile t: tw -> 3t+1, z -> 3t+2, reduce -> 3t+3;
        #          epilogue: recip -> 97, rep copy -> 98, alpha -> 99,
        #          at copy -> 100

        @block.sync
        def _(sync):
            sync.dma_start(
                v_sb.ap(), v.rearrange("(ko p) n -> p ko n", p=P)
            ).then_inc(s_v, 16)
            sync.dma_start(
                u_sb.ap(), u.rearrange("(ko p) n -> p ko n", p=P)
            ).then_inc(s_u, 16)
            sync.dma_start(w_sb.ap(), w_rep[:]).then_inc(s_w, 16)
            sync.dma_start(sel_sb.ap(), sel[:]).then_inc(s_sel, 16)
            sync.dma_start(sel2_sb.ap(), sel2[:]).then_inc(s_sel2, 16)
            sync.dma_start(id_sb.ap(), ident[:]).then_inc(s_id, 16)
            for s in range(NS):
                if s >= 2:
                    # x slot s%2 free once PE finished chunk s-2
                    sync.wait_ge(s_pe, 8 * (s - 1))
                sync.dma_start(
                    x_sb.ap()[:, s % 2, :, :],
                    xt3[:, :, s * MS * P : (s + 1) * MS * P],
                ).then_inc(s_x[s], 16)
            sync.wait_ge(s_dve, 100)
            sync.dma_start(out[:], at_sb.ap()).then_inc(s_out, 16)
            sync.wait_ge(s_out, 16)

        @block.tensor
        def _(tensor):
            tensor.wait_ge(s_v, 16)
            tensor.wait_ge(s_u, 16)
            for t in range(MT):
                s, q = divmod(t, MS)
                if q == 0:
                    tensor.wait_ge(s_x[s], 16)
                xq = x_sb.ap()[:, s % 2, :, :]
                if t >= 2:
                    tensor.wait_ge(s_act, 2 * (t - 2) + 1)
                for ko in range(KO):
                    mm = nc.tensor.matmul(
                        h_ps.ap()[:, t % 2, :],
                        xq[:, ko, q * P : (q + 1) * P],
                        v_sb.ap()[:, ko, :],
                        start=(ko == 0),
                        stop=(ko == KO - 1),
                    )
                mm.then_inc(s_pe, 1)
                if t >= 2:
                    tensor.wait_ge(s_act, 2 * (t - 2) + 2)
                for ko in range(KO):
                    mm = nc.tensor.matmul(
                        g_ps.ap()[:, t % 2, :],
                        xq[:, ko, q * P : (q + 1) * P],
                        u_sb.ap()[:, ko, :],
                        start=(ko == 0),
                        stop=(ko == KO - 1),
                    )
                mm.then_inc(s_pe, 1)
            # epilogue: per-batch softmax denominators
            tensor.wait_ge(s_sel, 16)
            tensor.wait_ge(s_act, 2 * MT + 1)  # exp/rowsum ready
            nc.tensor.matmul(
                gs_ps.ap(), sel_sb.ap(), rsum_sb.ap(), start=True, stop=True
            ).then_inc(s_pe, 1)  # -> 65
            tensor.wait_ge(s_sel2, 16)
            tensor.wait_ge(s_dve, 3 * MT + 1)  # recip ready
            nc.tensor.matmul(
                rep_ps.ap(), sel2_sb.ap(), recip_sb.ap(), start=True, stop=True
            ).then_inc(s_pe, 1)  # -> 66
            tensor.wait_ge(s_id, 16)
            tensor.wait_ge(s_dve, 3 * MT + 3)  # alpha ready
            nc.tensor.transpose(at_ps.ap(), alpha_sb.ap(), id_sb.ap()).then_inc(
                s_pe, 1
            )  # -> 67

        @block.scalar
        def _(scalar):
            for t in range(MT):
                scalar.wait_ge(s_pe, 2 * t + 1)
                if t >= 2:
                    scalar.wait_ge(s_dve, 3 * (t - 2) + 1)  # th slot free
                nc.scalar.activation(
                    th_sb.ap()[:, t % 2, :], h_ps.ap()[:, t % 2, :], AF.Tanh
                ).then_inc(s_act, 1)
                scalar.wait_ge(s_pe, 2 * t + 2)
                if t >= 2:
                    scalar.wait_ge(s_dve, 3 * (t - 2) + 2)  # sg slot free
                nc.scalar.activation(
                    sg_sb.ap()[:, t % 2, :], g_ps.ap()[:, t % 2, :], AF.Sigmoid
                ).then_inc(s_act, 1)
            # softmax numerators + row sums (no max-subtraction needed:
            # |score| <= sum|w| ~ 28, exp stays well inside fp32 range)
            scalar.wait_ge(s_dve, 3 * MT)  # S complete
            nc.scalar.activation(
                E_sb.ap(), S_sb.ap(), AF.Exp, accum_out=rsum_sb.ap()
            ).then_inc(s_act, 1)  # -> 65

        @block.vector
        def _(vector):
            vector.wait_ge(s_w, 16)
            for t in range(MT):
                vector.wait_ge(s_act, 2 * t + 1)
                nc.vector.tensor_tensor(
                    tw_sb.ap(), th_sb.ap()[:, t % 2, :], w_sb.ap(), ALU.mult
                ).then_inc(s_dve, 1)
                vector.wait_ge(s_act, 2 * t + 2)
                vector.wait_ge(s_dve, 3 * t + 1)  # tw RAW (same-engine order)
                nc.vector.tensor_tensor(
                    z_sb.ap(), tw_sb.ap(), sg_sb.ap()[:, t % 2, :], ALU.mult
                ).then_inc(s_dve, 1)
                vector.wait_ge(s_dve, 3 * t + 2)  # z RAW
                nc.vector.tensor_reduce(
                    S_sb.ap()[:, t : t + 1],
                    z_sb.ap(),
                    axis=mybir.AxisListType.X,
                    op=ALU.add,
                ).then_inc(s_dve, 1)
            # epilogue
            vector.wait_ge(s_pe, 2 * MT + 1)  # gs_ps ready
            nc.vector.reciprocal(recip_sb.ap(), gs_ps.ap()).then_inc(s_dve, 1)  # 97
            vector.wait_ge(s_pe, 2 * MT + 2)  # rep_ps ready
            nc.vector.tensor_copy(rep_sb.ap(), rep_ps.ap()).then_inc(s_dve, 1)  # 98
            vector.wait_ge(s_act, 2 * MT + 1)  # E ready
            vector.wait_ge(s_dve, 3 * MT + 2)  # rep_sb RAW
            nc.vector.tensor_scalar_mul(
                alpha_sb.ap(), E_sb.ap(), rep_sb.ap()
            ).then_inc(s_dve, 1)  # 99
            vector.wait_ge(s_pe, 2 * MT + 3)  # at_ps ready
            nc.vector.tensor_copy(at_sb.ap(), at_ps.ap()).then_inc(s_dve, 1)  # 100

    return nc


def _host_inputs(x, v, u, w):
    """Build the per-core input maps (host-side shard + layout prep)."""
    x = np.asarray(x, dtype=np.float32)
    v = np.ascontiguousarray(np.asarray(v, dtype=np.float32))
    u = np.ascontiguousarray(np.asarray(u, dtype=np.float32))
    w = np.asarray(w, dtype=np.float32).reshape(L_DIM)

    w_rep = np.ascontiguousarray(np.broadcast_to(w, (P, L_DIM)))
    sel = (np.arange(P)[:, None] % B_LOC == np.arange(B_LOC)[None, :]).astype(
        np.float32
    )
    sel2 = np.ascontiguousarray(sel.T)
    ident = np.eye(P, dtype=np.float32)

    common = {
        "v": v,
        "u": u,
        "w_rep": w_rep,
        "sel": sel,
        "sel2": sel2,
        "ident": ident,
    }
    in_maps = []
    for c in range(N_CORES):
        xc = x[:, c * B_LOC : (c + 1) * B_LOC, :].reshape(M, IN_DIM)
        xtc = np.ascontiguousarray(xc.T)  # [IN_DIM, M]
        in_maps.append({"xt": xtc, **common})
    return in_maps


def kernel(x, v, u, w):
    from concourse.bass_utils import run_bass_kernel_spmd

    if "nc" not in _CACHE:
        _CACHE["nc"] = _build_bass()
    nc = _CACHE["nc"]

    in_maps = _host_inputs(x, v, u, w)
    res = run_bass_kernel_spmd(nc, in_maps, core_ids=list(range(N_CORES)))
    _CACHE["last_result"] = res

    parts = []
    for c in range(N_CORES):
        a = res.results[c]["out"]  # [32, 128], flat index = m = i*16 + b_loc
        parts.append(a.reshape(N_INST, B_LOC))
    full = np.concatenate(parts, axis=1)[:, :, None]
    return np.ascontiguousarray(full.astype(np.float32))


# revision 16
# speedup vs baseline: 1.0471x; 1.0471x over previous
"""Trainium2 Bass kernel for the gated-attention MIL pooling layer.

Computes, for x:[256,128,1024], v,u:[1024,512], w:[512,1]:
    h = tanh(x @ v); g = sigmoid(x @ u)
    scores = (h*g) @ w                      # [256,128,1]
    alpha  = softmax(scores, axis=0)        # over the 256 instances

Sharding: data-parallel over the batch axis (128 -> 16 per core, 8 cores).
Each core handles a [4096,1024]x[1024,512] matmul pair + a local softmax
(softmax is over instances, which live entirely on one core).

x is transposed host-side to [in_dim, m] so the Bass kernel can feed the
PE array without on-chip transposes (the contraction dim must sit on SBUF
partitions).  Matmuls run as float32r (full PE rate at moving dim >= 256,
~fp32 precision).

Written in raw Bass (explicit per-engine programs + semaphores): the
walrus build in this container rejects instructions carrying more than
one attached semaphore wait, which rules out Tile-generated sync.  All
waits here are standalone wait_ge instructions.

Startup is DMA-bound, so the first chunk is streamed per-k-subtile:
(v[ko], x0[ko], u[ko]) DMA triples feed matmuls for all 8 PSUM
accumulation groups (4 h + 4 g) of chunk 0 as the pieces land, instead
of waiting ~29us for all weights before the first matmul.
"""

import numpy as np

N_INST, BATCH, IN_DIM, L_DIM = 256, 128, 1024, 512
N_CORES = 8
B_LOC = BATCH // N_CORES            # 16 batch elements per core
M = N_INST * B_LOC                  # 4096 rows per core
P = 128                             # SBUF partitions
KO = IN_DIM // P                    # 8 contraction subtiles
MT = M // P                         # 32 m-tiles per core
MS = 4                              # m-tiles per x DMA chunk
NS = MT // MS                       # 8 DMA chunks

_CACHE = {}


def _build_bass():
    from contextlib import ExitStack

    import concourse.bass as bass
    import concourse.mybir as mybir

    f32 = mybir.dt.float32
    f32r = mybir.dt.float32r
    AF = mybir.ActivationFunctionType
    ALU = mybir.AluOpType

    nc = bass.Bass(
        trn_type="TRN2",
        target_bir_lowering=False,
        debug=False,
        enable_asserts=False,
    )

    xt = nc.dram_tensor("xt", [IN_DIM, M], f32r, kind="ExternalInput").ap()
    v = nc.dram_tensor("v", [IN_DIM, L_DIM], f32r, kind="ExternalInput").ap()
    u = nc.dram_tensor("u", [IN_DIM, L_DIM], f32r, kind="ExternalInput").ap()
    w_rep = nc.dram_tensor("w_rep", [P, L_DIM], f32, kind="ExternalInput").ap()
    sel = nc.dram_tensor("sel", [P, B_LOC], f32, kind="ExternalInput").ap()
    sel2 = nc.dram_tensor("sel2", [B_LOC, P], f32, kind="ExternalInput").ap()
    ident = nc.dram_tensor("ident", [P, P], f32, kind="ExternalInput").ap()
    out = nc.dram_tensor("out", [MT, P], f32, kind="ExternalOutput").ap()

    xt3 = xt.rearrange("(ko p) m -> p ko m", p=P)
    v3 = v.rearrange("(ko p) n -> p ko n", p=P)
    u3 = u.rearrange("(ko p) n -> p ko n", p=P)

    # Activation-semaphore tick after tanh/sigmoid of tile t (chunk 0 runs
    # all four tanh before the sigmoids; steady chunks alternate).
    def act_tanh(t):
        return t + 1 if t < MS else 2 * t + 1

    def act_sig(t):
        return MS + 1 + t if t < MS else 2 * t + 2

    ctx = ExitStack()
    with ctx:
        v_sb = ctx.enter_context(nc.sbuf_tensor("v_sb", [P, KO, L_DIM], f32r))
        u_sb = ctx.enter_context(nc.sbuf_tensor("u_sb", [P, KO, L_DIM], f32r))
        w_sb = ctx.enter_context(nc.sbuf_tensor("w_sb", [P, L_DIM], f32))
        sel_sb = ctx.enter_context(nc.sbuf_tensor("sel_sb", [P, B_LOC], f32))
        sel2_sb = ctx.enter_context(nc.sbuf_tensor("sel2_sb", [B_LOC, P], f32))
        id_sb = ctx.enter_context(nc.sbuf_tensor("id_sb", [P, P], f32))
        x_sb = ctx.enter_context(nc.sbuf_tensor("x_sb", [P, 2, KO, MS * P], f32r))
        th_sb = ctx.enter_context(nc.sbuf_tensor("th_sb", [P, MS, L_DIM], f32))
        sg_sb = ctx.enter_context(nc.sbuf_tensor("sg_sb", [P, MS, L_DIM], f32))
        tw_sb = ctx.enter_context(nc.sbuf_tensor("tw_sb", [P, L_DIM], f32))
        z_sb = ctx.enter_context(nc.sbuf_tensor("z_sb", [P, L_DIM], f32))
        S_sb = ctx.enter_context(nc.sbuf_tensor("S_sb", [P, MT], f32))
        E_sb = ctx.enter_context(nc.sbuf_tensor("E_sb", [P, MT], f32))
        rsum_sb = ctx.enter_context(nc.sbuf_tensor("rsum_sb", [P, 1], f32))
        recip_sb = ctx.enter_context(nc.sbuf_tensor("recip_sb", [B_LOC, 1], f32))
        rep_sb = ctx.enter_context(nc.sbuf_tensor("rep_sb", [P, 1], f32))
        alpha_sb = ctx.enter_context(nc.sbuf_tensor("alpha_sb", [P, MT], f32))
        at_sb = ctx.enter_context(nc.sbuf_tensor("at_sb", [MT, P], f32))

        # All 8 PSUM banks: 4 h accumulation groups + 4 g groups (slot t%4).
        h_ps = ctx.enter_context(nc.psum_tensor("h_ps", [P, MS, L_DIM], f32))
        g_ps = ctx.enter_context(nc.psum_tensor("g_ps", [P, MS, L_DIM], f32))
        # Epilogue PSUM aliases h banks (dead by then; gated on s_act >= exp).
        gs_ps = h_ps.ap()[:B_LOC, 0, :1]     # [16, 1]
        rep_ps = h_ps.ap()[:, 1, :1]         # [128, 1]
        at_ps = h_ps.ap()[:MT, 2, :P]        # [32, 128]

        s_v = [ctx.enter_context(nc.semaphore(f"s_v{k}")) for k in range(KO)]
        s_u = [ctx.enter_context(nc.semaphore(f"s_u{k}")) for k in range(KO)]
        s_x0 = [ctx.enter_context(nc.semaphore(f"s_x0k{k}")) for k in range(KO)]
        s_x1 = [ctx.enter_context(nc.semaphore(f"s_x1k{k}")) for k in range(KO)]
        s_w = ctx.enter_context(nc.semaphore("s_w"))
        s_sel = ctx.enter_context(nc.semaphore("s_sel"))
        s_sel2 = ctx.enter_context(nc.semaphore("s_sel2"))
        s_id = ctx.enter_context(nc.semaphore("s_id"))
        s_x = [ctx.enter_context(nc.semaphore(f"s_x{i}")) for i in range(NS)]
        s_out = ctx.enter_context(nc.semaphore("s_out"))
        s_pe = ctx.enter_context(nc.semaphore("s_pe"))
        s_act = ctx.enter_context(nc.semaphore("s_act"))
        s_dve = ctx.enter_context(nc.semaphore("s_dve"))

        block = ctx.enter_context(nc.Block())

        # Tick conventions:
        #   s_pe : +1 per finished accumulation group. Chunk 0: h0..h3 ->
        #          1..4, g0..g3 -> 5..8; steady tile t: h -> 2t+1, g -> 2t+2.
        #          Epilogue matmuls -> 65, 66, 67.
        #   s_act: see act_tanh/act_sig above; exp -> 65.
        #   s_dve: tile t: tw -> 3t+1, z -> 3t+2, reduce -> 3t+3;
        #          epilogue: recip 97, rep copy 98, alpha 99, at copy 100.

        @block.sync
        def _(sync):
            # chunk-0 startup stream: (v, x0, u) per k-subtile
            for ko in range(KO):
                sync.dma_start(
                    v_sb.ap()[:, ko, :], v3[:, ko, :]
                ).then_inc(s_v[ko], 16)
                sync.dma_start(
                    x_sb.ap()[:, 0, ko, :], xt3[:, ko, : MS * P]
                ).then_inc(s_x0[ko], 16)
                sync.dma_start(
                    u_sb.ap()[:, ko, :], u3[:, ko, :]
                ).then_inc(s_u[ko], 16)
            sync.dma_start(w_sb.ap(), w_rep[:]).then_inc(s_w, 16)
            # chunk 1, also per-ko so tile 4 can start as pieces land
            for ko in range(KO):
                sync.dma_start(
                    x_sb.ap()[:, 1, ko, :], xt3[:, ko, MS * P : 2 * MS * P]
                ).then_inc(s_x1[ko], 16)
            sync.dma_start(sel_sb.ap(), sel[:]).then_inc(s_sel, 16)
            sync.dma_start(sel2_sb.ap(), sel2[:]).then_inc(s_sel2, 16)
            sync.dma_start(id_sb.ap(), ident[:]).then_inc(s_id, 16)
            for s in range(2, NS):
                # x slot s%2 free once PE finished chunk s-2
                sync.wait_ge(s_pe, 8 * (s - 1))
                sync.dma_start(
                    x_sb.ap()[:, s % 2, :, :],
                    xt3[:, :, s * MS * P : (s + 1) * MS * P],
                ).then_inc(s_x[s], 16)
            sync.wait_ge(s_dve, 3 * MT + 4)
            sync.dma_start(out[:], at_sb.ap()).then_inc(s_out, 16)
            sync.wait_ge(s_out, 16)

        @block.tensor
        def _(tensor):
            # ---- chunk 0: ko-outer over all 8 psum accumulation groups ----
            x0 = x_sb.ap()[:, 0, :, :]
            for ko in range(KO):
                tensor.wait_ge(s_v[ko], 16)
                tensor.wait_ge(s_x0[ko], 16)
                for t in range(MS):
                    mm = nc.tensor.matmul(
                        h_ps.ap()[:, t, :],
                        x0[:, ko, t * P : (t + 1) * P],
                        v_sb.ap()[:, ko, :],
                        start=(ko == 0),
                        stop=(ko == KO - 1),
                    )
                    if ko == KO - 1:
                        mm.then_inc(s_pe, 1)  # ticks 1..4
                tensor.wait_ge(s_u[ko], 16)
                for t in range(MS):
                    mm = nc.tensor.matmul(
                        g_ps.ap()[:, t, :],
                        x0[:, ko, t * P : (t + 1) * P],
                        u_sb.ap()[:, ko, :],
                        start=(ko == 0),
                        stop=(ko == KO - 1),
                    )
                    if ko == KO - 1:
                        mm.then_inc(s_pe, 1)  # ticks 5..8
            # ---- steady chunks ----
            for t in range(MS, MT):
                s, q = divmod(t, MS)
                xq = x_sb.ap()[:, s % 2, :, :]
                # h bank t%4 free once tanh(t-4) done
                tensor.wait_ge(s_act, act_tanh(t - MS))
                if q == 0 and s > 1:
                    tensor.wait_ge(s_x[s], 16)
                for ko in range(KO):
                    if q == 0 and s == 1:
                        tensor.wait_ge(s_x1[ko], 16)
                    mm = nc.tensor.matmul(
                        h_ps.ap()[:, t % MS, :],
                        xq[:, ko, q * P : (q + 1) * P],
                        v_sb.ap()[:, ko, :],
                        start=(ko == 0),
                        stop=(ko == KO - 1),
                    )
                mm.then_inc(s_pe, 1)  # tick 2t+1
                tensor.wait_ge(s_act, act_sig(t - MS))
                for ko in range(KO):
                    mm = nc.tensor.matmul(
                        g_ps.ap()[:, t % MS, :],
                        xq[:, ko, q * P : (q + 1) * P],
                        u_sb.ap()[:, ko, :],
                        start=(ko == 0),
                        stop=(ko == KO - 1),
                    )
                mm.then_inc(s_pe, 1)  # tick 2t+2
            # ---- epilogue: per-batch softmax denominators ----
            tensor.wait_ge(s_sel, 16)
            tensor.wait_ge(s_act, 2 * MT + 1)  # exp/rowsum ready; h banks dead
            nc.tensor.matmul(
                gs_ps, sel_sb.ap(), rsum_sb.ap(), start=True, stop=True
            ).then_inc(s_pe, 1)  # -> 65
            tensor.wait_ge(s_sel2, 16)
            tensor.wait_ge(s_dve, 3 * MT + 1)  # recip ready
            nc.tensor.matmul(
                rep_ps, sel2_sb.ap(), recip_sb.ap(), start=True, stop=True
            ).then_inc(s_pe, 1)  # -> 66
            tensor.wait_ge(s_id, 16)
            tensor.wait_ge(s_dve, 3 * MT + 3)  # alpha ready
            nc.tensor.transpose(at_ps, alpha_sb.ap(), id_sb.ap()).then_inc(
                s_pe, 1
            )  # -> 67

        @block.scalar
        def _(scalar):
            # chunk 0: four tanh (as h groups finish), then four sigmoid
            for t in range(MS):
                scalar.wait_ge(s_pe, t + 1)
                nc.scalar.activation(
                    th_sb.ap()[:, t, :], h_ps.ap()[:, t, :], AF.Tanh
                ).then_inc(s_act, 1)
            for t in range(MS):
                scalar.wait_ge(s_pe, MS + 1 + t)
                nc.scalar.activation(
                    sg_sb.ap()[:, t, :], g_ps.ap()[:, t, :], AF.Sigmoid
                ).then_inc(s_act, 1)
            for t in range(MS, MT):
                scalar.wait_ge(s_pe, 2 * t + 1)
                scalar.wait_ge(s_dve, 3 * (t - MS) + 1)  # th slot free
                nc.scalar.activation(
                    th_sb.ap()[:, t % MS, :], h_ps.ap()[:, t % MS, :], AF.Tanh
                ).then_inc(s_act, 1)
                scalar.wait_ge(s_pe, 2 * t + 2)
                scalar.wait_ge(s_dve, 3 * (t - MS) + 2)  # sg slot free
                nc.scalar.activation(
                    sg_sb.ap()[:, t % MS, :], g_ps.ap()[:, t % MS, :], AF.Sigmoid
                ).then_inc(s_act, 1)
            # softmax numerators + row sums (no max-subtraction needed:
            # |score| <= sum|w| ~ 28, exp stays well inside fp32 range)
            scalar.wait_ge(s_dve, 3 * MT)  # S complete
            nc.scalar.activation(
                E_sb.ap(), S_sb.ap(), AF.Exp, accum_out=rsum_sb.ap()
            ).then_inc(s_act, 1)  # -> 65

        @block.vector
        def _(vector):
            vector.wait_ge(s_w, 16)
            for t in range(MT):
                vector.wait_ge(s_act, act_tanh(t))
                nc.vector.tensor_tensor(
                    tw_sb.ap(), th_sb.ap()[:, t % MS, :], w_sb.ap(), ALU.mult
                ).then_inc(s_dve, 1)
                vector.wait_ge(s_act, act_sig(t))
                vector.wait_ge(s_dve, 3 * t + 1)  # tw RAW (same-engine order)
                nc.vector.tensor_tensor(
                    z_sb.ap(), tw_sb.ap(), sg_sb.ap()[:, t % MS, :], ALU.mult
                ).then_inc(s_dve, 1)
                vector.wait_ge(s_dve, 3 * t + 2)  # z RAW
                nc.vector.tensor_reduce(
                    S_sb.ap()[:, t : t + 1],
                    z_sb.ap(),
                    axis=mybir.AxisListType.X,
                    op=ALU.add,
                ).then_inc(s_dve, 1)
            # epilogue
            vector.wait_ge(s_pe, 2 * MT + 1)  # gs_ps ready
            nc.vector.reciprocal(recip_sb.ap(), gs_ps).then_inc(s_dve, 1)  # 97
            vector.wait_ge(s_pe, 2 * MT + 2)  # rep_ps ready
            nc.vector.tensor_copy(rep_sb.ap(), rep_ps).then_inc(s_dve, 1)  # 98
            vector.wait_ge(s_act, 2 * MT + 1)  # E ready
            vector.wait_ge(s_dve, 3 * MT + 2)  # rep_sb RAW
            nc.vector.tensor_scalar_mul(
                alpha_sb.ap(), E_sb.ap(), rep_sb.ap()
            ).then_inc(s_dve, 1)  # 99
            vector.wait_ge(s_pe, 2 * MT + 3)  # at_ps ready
            nc.vector.tensor_copy(at_sb.ap(), at_ps).then_inc(s_dve, 1)  # 100

    return nc


def _host_inputs(x, v, u, w):
    """Build the per-core input maps (host-side shard + layout prep)."""
    x = np.asarray(x, dtype=np.float32)
    v = np.ascontiguousarray(np.asarray(v, dtype=np.float32))
    u = np.ascontiguousarray(np.asarray(u, dtype=np.float32))
    w = np.asarray(w, dtype=np.float32).reshape(L_DIM)

    w_rep = np.ascontiguousarray(np.broadcast_to(w, (P, L_DIM)))
    sel = (np.arange(P)[:, None] % B_LOC == np.arange(B_LOC)[None, :]).astype(
        np.float32
    )
    sel2 = np.ascontiguousarray(sel.T)
    ident = np.eye(P, dtype=np.float32)

    common = {
        "v": v,
        "u": u,
        "w_rep": w_rep,
        "sel": sel,
        "sel2": sel2,
        "ident": ident,
    }
    in_maps = []
    for c in range(N_CORES):
        xc = x[:, c * B_LOC : (c + 1) * B_LOC, :].reshape(M, IN_DIM)
        xtc = np.ascontiguousarray(xc.T)  # [IN_DIM, M]
        in_maps.append({"xt": xtc, **common})
    return in_maps


def kernel(x, v, u, w):
    from concourse.bass_utils import run_bass_kernel_spmd

    if "nc" not in _CACHE:
        _CACHE["nc"] = _build_bass()
    nc = _CACHE["nc"]

    in_maps = _host_inputs(x, v, u, w)
    res = run_bass_kernel_spmd(nc, in_maps, core_ids=list(range(N_CORES)))
    _CACHE["last_result"] = res

    parts = []
    for c in range(N_CORES):
        a = res.results[c]["out"]  # [32, 128], flat index = m = i*16 + b_loc
        parts.append(a.reshape(N_INST, B_LOC))
    full = np.concatenate(parts, axis=1)[:, :, None]
    return np.ascontiguousarray(full.astype(np.float32))
